# revision 76
# baseline (speedup 1.0000x reference)
"""Multi-head self-attention (B=4, T=2048, C=1024, 16 heads x hd=64) on 8
Trainium2 NeuronCores.

Sharding: tensor-parallel over heads — each core owns 2 heads (128 of the
1024 channels): its slices of Wq/Wk/Wv rows and Wo columns. Every core reads
the full x (transposed + bf16-cast on host), computes Q^T/K^T (channel-major)
and V (token-major) for its heads, runs attention entirely from SBUF, then
produces a rank-128 partial of the output projection. The 8 partials are
summed on host (+ bo).

Per-core dataflow (all matmuls bf16 in / fp32 PSUM accumulate):
  phase 1: Q^T = Wq_c @ x^T (+bq), K^T = Wk_c @ x^T (bk dropped — it only
           shifts every score in a softmax row by a constant), V = x @ Wv_c^T
           token-major with a ones column appended per head (denominator
           trick) and bv folded into V (softmax weights sum to 1, so adding
           bv to every V row adds exactly bv to the output).
  phase 2: per (batch, 512-query block): S^T [128k, 1024(2 k-tiles)] per head
           via K^T-stationary matmuls (contraction d=64), one exp per k-tile
           pair on ScalarE (scale=1/8 folded in) -> P^T bf16, then
           O^T[65,512] += [V|1]^T P^T accumulated over k with K=128 matmuls.
           Softmax denominator lands in row 64; both heads' reciprocals are
           broadcast over 64 partitions with a single selector matmul on the
           PE (no DMA round-trips), then ot = pv * recip on VectorE.
  phase 3: partial_out[128 rows, 512] = O^T-slice-stationary matmuls against
           Wo_c^T; fp16 partials DMA'd out.

Scheduling: a global FIFO of small (~0.5-2us) filler closures is drained
inside phase 2's ACT-bound loops. Per global query-block G the fillers are
K/V projections for G+4, the Q projection for G+1 (Q is only needed by its
own query block, so it rides one block ahead), and output-projection tiles
of G-4 (plus G-1 during the last batch, which has no projections left).
PSUM->SBUF drains for V and phase 3 run on GpSimd (otherwise idle); x and
weight loads are single-dispatch DMAs (packets stripe across all 16 DMA
engines regardless, and each dispatch costs ~780ns of Sync-engine time).
"""
import json

import numpy as np
import ml_dtypes

import concourse.bass as bass
import concourse.mybir as mybir
import concourse.tile as tile
from concourse.bass_utils import run_bass_kernel_spmd

bf16 = ml_dtypes.bfloat16
dt = mybir.dt

EMB = 1024
HEADS = 16
HD = 64
B = 4
T = 2048
R = B * T            # 8192 rows
NCORES = 8
F = EMB // NCORES    # 128 channels (2 heads) per core
NH = F // HD         # 2 heads per core
NKC = EMB // 128     # 8 contraction chunks for projections
NQB = T // 512       # 4 query blocks per batch
NJP = T // 256       # 8 k-tile PAIRS per batch
NG = R // 512        # 16 global query blocks
G = R // 128         # 64 global row/key tiles
VW = HD + 1          # 65: V head slice + ones column


# ---------------------------------------------------------------------------
# walrus in this container accepts only ONE sync-wait per instruction; split
# extra waits onto same-engine NoOps at BIR-serialization time.
_orig_to_json_bytes = bass.Bass.to_json_bytes


def _split_waits(data: bytes) -> bytes:
    d = json.loads(data)
    changed = False
    for f in d.get("functions", []):
        for blk in f.get("blocks", []):
            out = []
            for inst in blk.get("instructions", []):
                si = inst.get("sync_info")
                waits = (si or {}).get("on_wait") or []
                if len(waits) > 1:
                    changed = True
                    for i, w in enumerate(waits[:-1]):
                        out.append({
                            "debug": inst.get("debug", 0),
                            "engine": inst["engine"],
                            "ins": [], "outs": [],
                            "name": f"{inst['name']}_w{i}",
                            "opcode": "NoOp",
                            "sync_info": {"on_update": [], "on_wait": [w]},
                            "text_hint": "wait_split",
                        })
                    si["on_wait"] = waits[-1:]
                out.append(inst)
            blk["instructions"] = out
    return json.dumps(d).encode() if changed else data


def _to_json_bytes(self, *a, **k):
    return _split_waits(_orig_to_json_bytes(self, *a, **k))


bass.Bass.to_json_bytes = _to_json_bytes
# ---------------------------------------------------------------------------


def build_bass() -> bass.Bass:
    nc = bass.Bass()
    xt_ext = nc.declare_dram_parameter("xt", [EMB, R], dt.bfloat16, isOutput=False)
    wq_ext = nc.declare_dram_parameter("wq", [EMB, F], dt.bfloat16, isOutput=False)
    wk_ext = nc.declare_dram_parameter("wk", [EMB, F], dt.bfloat16, isOutput=False)
    wv_ext = nc.declare_dram_parameter("wv", [EMB, F], dt.bfloat16, isOutput=False)
    wo_ext = nc.declare_dram_parameter("wo", [F, EMB], dt.bfloat16, isOutput=False)
    bq_ext = nc.declare_dram_parameter("bq", [F, 1], dt.float32, isOutput=False)
    bv_ext = nc.declare_dram_parameter("bv", [1, F], dt.float32, isOutput=False)
    out_ext = nc.declare_dram_parameter("out", [R, EMB], dt.float16, isOutput=True)

    Exp = mybir.ActivationFunctionType.Exp

    with tile.TileContext(nc) as tc:
        with (
            tc.tile_pool(name="const", bufs=1) as cp,
            tc.tile_pool(name="res", bufs=1) as res,
            tc.tile_pool(name="xt", bufs=1) as xp,
            tc.tile_pool(name="pt", bufs=6) as ptp,
            tc.tile_pool(name="norm", bufs=2) as npl,
            tc.tile_pool(name="osb", bufs=3) as op,
            tc.tile_pool(name="ps", bufs=1, space="PSUM") as ps,
        ):
            # --- constants ---
            wq_sb = cp.tile([128, EMB], dt.bfloat16, tag="wq")
            wk_sb = cp.tile([128, EMB], dt.bfloat16, tag="wk")
            wv_sb = cp.tile([128, EMB], dt.bfloat16, tag="wv")
            wo_sb = cp.tile([128, EMB], dt.bfloat16, tag="wo")
            bq_sb = cp.tile([F, 1], dt.float32, tag="bq")
            bvb_sb = cp.tile([128, F], dt.float32, tag="bvb")
            ones_sb = cp.tile([1, HD], dt.bfloat16, tag="ones")

            def _wload(ext, tile_sb):
                nc.sync.dma_start(
                    tile_sb[:].rearrange("p (kc f) -> p kc f", f=F),
                    ext[:].rearrange("(kc p) f -> p kc f", p=128),
                )

            # --- residents ---
            qt_sb = res.tile([F, R], dt.bfloat16, tag="qt")
            kt_sb = res.tile([F, R], dt.bfloat16, tag="kt")
            ot_sb = res.tile([F, R], dt.bfloat16, tag="ot")
            va_sb = res.tile([128, G * NH * VW], dt.bfloat16, tag="va")

            # ---- x loads: two dispatches per 512-row block (the split lets
            # the first projection matmuls start after half the data) ----
            def load_x(rb, tag, bufs=4):
                xts = []
                for half in range(2):
                    xt = xp.tile([128, 4 * 512], dt.bfloat16,
                                 tag=f"{tag}{half}", bufs=bufs,
                                 name=f"{tag}{half}_{rb}")
                    nc.sync.dma_start(
                        xt[:].rearrange("p (kc f) -> p kc f", f=512),
                        xt_ext[half * 512:half * 512 + 512,
                               rb * 512:rb * 512 + 512]
                        .rearrange("(kc p) f -> p kc f", p=128),
                    )
                    xts.append(xt)
                return xts

            def xsl(xts, kc, lo, hi):
                base = (kc % 4) * 512
                return xts[kc // 4][:, base + lo:base + hi]

            # ---- projection emitters ----
            def p1_qk(rb, xt, w_sb, dst_sb, bias):
                r0 = rb * 512
                acc = ps.tile([128, 512], dt.float32, tag="pp", bufs=2,
                              name=f"prj_{rb}_{id(w_sb)}")
                for kc in range(NKC):
                    nc.tensor.matmul(
                        acc[:], w_sb[:, kc * F:(kc + 1) * F],
                        xsl(xt, kc, 0, 512),
                        start=(kc == 0), stop=(kc == NKC - 1),
                    )
                if bias is not None:
                    nc.vector.tensor_scalar_add(
                        dst_sb[:, r0:r0 + 512], acc[:], bias[:])
                else:
                    nc.vector.tensor_copy(dst_sb[:, r0:r0 + 512], acc[:])

            def p1_v(rb, xt, sub):
                g = rb * 4 + sub
                acc = ps.tile([128, F], dt.float32, tag="pp", bufs=2,
                              name=f"vprj_{g}")
                for kc in range(NKC):
                    nc.tensor.matmul(
                        acc[:],
                        xsl(xt, kc, sub * 128, (sub + 1) * 128),
                        wv_sb[:, kc * F:(kc + 1) * F],
                        start=(kc == 0), stop=(kc == NKC - 1),
                    )
                dst = va_sb[:, g * NH * VW:(g + 1) * NH * VW].rearrange(
                    "p (h d) -> p h d", d=VW
                )[:, :, 0:HD]
                nc.vector.tensor_add(
                    dst, acc[:].rearrange("p (h d) -> p h d", d=HD),
                    bvb_sb[:].rearrange("p (h d) -> p h d", d=HD),
                )

            # ---- phase-3 emitter (one 128-row tile); out DMA dispatched off
            # the otherwise-idle GpSimd DGE queue ----
            def p3_tile(g):
                o_sb = op.tile([128, EMB], dt.float16, tag="osb", name=f"o_{g}")
                for ch in range(2):
                    o_ps = ps.tile([128, 512], dt.float32, tag="pp", bufs=2,
                                   name=f"ops_{g}_{ch}")
                    nc.tensor.matmul(
                        o_ps[:],
                        ot_sb[:, g * 128:(g + 1) * 128],
                        wo_sb[:, ch * 512:(ch + 1) * 512],
                        start=True, stop=True,
                    )
                    nc.vector.tensor_copy(o_sb[:, ch * 512:(ch + 1) * 512], o_ps[:])
                nc.gpsimd.dma_start(out_ext[g * 128:(g + 1) * 128, :], o_sb[:])

            # ---- global filler FIFO ----
            # Closures marked is_load=True emit no PE work (DMA dispatch
            # only); fill() doesn't count them toward its quota so a slot
            # between two PE-gating points always gets real PE filler.
            fifo = []

            def load_closure(fn):
                fn.is_load = True
                return fn

            def fill(n=1):
                for _ in range(n):
                    if fifo:
                        fifo.pop(0)()

            # ---- phase-2 q-block with interleaved fillers ----
            def p2_qblock(b, qb, pull2=False, last=False):
                q0 = b * T + qb * 512
                pvs = {h: ps.tile([VW, 512], dt.float32, tag="pv", bufs=2,
                                  name=f"pv_{b}_{qb}_{h}")
                       for h in range(NH)}
                pts = {}

                def emit_st(jp, midfill=False):
                    k0 = b * T + jp * 256
                    for h in range(NH):
                        st = ps.tile([128, 1024], dt.float32, tag="st", bufs=2,
                                     name=f"st_{b}_{qb}_{jp}_{h}")
                        for half in range(2):
                            nc.tensor.matmul(
                                st[:, half * 512:(half + 1) * 512],
                                kt_sb[h * HD:(h + 1) * HD,
                                      k0 + half * 128:k0 + (half + 1) * 128],
                                qt_sb[h * HD:(h + 1) * HD, q0:q0 + 512],
                                start=True, stop=True,
                            )
                        pt = ptp.tile([128, 1024], dt.bfloat16, tag="pt",
                                      name=f"pt_{b}_{qb}_{jp}_{h}")
                        nc.scalar.activation(pt[:], st[:], Exp, scale=0.125)
                        pts[(jp, h)] = pt
                        if midfill and h == 0:
                            fill(1)

                def emit_pv(jp):
                    g0 = b * NJP * 2 + jp * 2
                    for h in range(NH):
                        pt = pts.pop((jp, h))
                        for half in range(2):
                            g = g0 + half
                            va = va_sb[:, g * NH * VW + h * VW:
                                       g * NH * VW + (h + 1) * VW]
                            nc.tensor.matmul(
                                pvs[h][:], va[:],
                                pt[:, half * 512:(half + 1) * 512],
                                start=(jp == 0 and half == 0),
                                stop=(jp == NJP - 1 and half == 1),
                            )

                # ~13 pulls per q-block matches the ~13 closures enqueued per
                # q-block: pulling faster drains the FIFO mid-block and
                # leaves later jps with no PE filler, stalling S on the
                # st-slot WAR (ACT of the same head one jp earlier).
                for jp in range(NJP):
                    emit_st(jp)
                    fill(2 if (jp < 2 or pull2) else (1 if jp % 2 == 0 else 0))
                    if jp > 1:
                        emit_pv(jp - 2)
                        fill(1)
                emit_pv(NJP - 2)
                fill(1)
                emit_pv(NJP - 1)
                # ---- normalize. The m copies (rows 0:65, incl the
                # denominator) release the pv PSUM banks early so the next
                # q-block's PV accumulation never waits on this chain. The
                # denominator rows are DMA-reshaped to [128,4] so the DVE
                # reciprocal runs 128 lanes wide (~0.2us, vs 3.3us on one
                # lane), DMA'd back to [1,512] bf16, broadcast over 64
                # partitions with a K=1 ones matmul, then ot = m * bps.
                # Everything past the m copies is slack: ot is only read by
                # p3_tile fillers 4 q-blocks later. ----
                if last:
                    # tail: 1/denom = exp(-ln denom) on the (now idle)
                    # ScalarE — shorter critical path than the DMA reshape
                    bps = ps.tile([128, 512], dt.float32, tag="pp", bufs=2,
                                  name=f"bps_{b}_{qb}")
                    ms = []
                    for h in range(NH):
                        m = npl.tile([VW, 512], dt.float32, tag="m", bufs=4,
                                     name=f"m_{b}_{qb}_{h}")
                        nc.vector.tensor_copy(m[:], pvs[h][:])
                        ms.append(m)
                    for h in range(NH):
                        ld = npl.tile([1, 512], dt.float32, tag="ld",
                                      name=f"ld_{b}_{qb}_{h}")
                        nc.scalar.activation(
                            ld[:], ms[h][HD:VW, :],
                            mybir.ActivationFunctionType.Ln)
                        rcb = npl.tile([1, 512], dt.bfloat16, tag="rcb",
                                       bufs=4, name=f"rcbL_{b}_{qb}_{h}")
                        nc.scalar.activation(
                            rcb[:], ld[:],
                            mybir.ActivationFunctionType.Exp, scale=-1.0)
                        nc.tensor.matmul(
                            bps[h * HD:(h + 1) * HD, :], ones_sb[:], rcb[:],
                            start=True, stop=True,
                        )
                    for h in range(NH):
                        nc.vector.tensor_mul(
                            ot_sb[h * HD:(h + 1) * HD, q0:q0 + 512],
                            ms[h][0:HD, :], bps[h * HD:(h + 1) * HD, :],
                        )
                    return
                ms = []
                d4 = npl.tile([128, 2 * 4], dt.float32, tag="d4",
                              name=f"d4_{b}_{qb}")
                for h in range(NH):
                    m = npl.tile([VW, 512], dt.float32, tag="m", bufs=4,
                                 name=f"m_{b}_{qb}_{h}")
                    nc.vector.tensor_copy(m[:], pvs[h][:])
                    ms.append(m)
                    nc.sync.dma_start(
                        d4[:, h * 4:(h + 1) * 4]
                        .rearrange("p (a c) -> p a c", c=4),
                        m[HD:VW, :].rearrange("p (a c) -> p a c", c=4),
                    )
                r4 = npl.tile([128, 2 * 4], dt.float32, tag="r4",
                              name=f"r4_{b}_{qb}")
                nc.vector.reciprocal(r4[:], d4[:])
                r4b = npl.tile([128, 2 * 4], dt.bfloat16, tag="r4b",
                               name=f"r4b_{b}_{qb}")
                with nc.allow_low_precision(reason="1/denom bf16: 2^-9 rel"):
                    nc.vector.tensor_copy(r4b[:], r4[:])
                rcbs = []
                for h in range(NH):
                    rcb = npl.tile([1, 512], dt.bfloat16, tag="rcb", bufs=4,
                                   name=f"rcb_{b}_{qb}_{h}")
                    nc.sync.dma_start(
                        rcb[:].rearrange("p (a c) -> p a c", c=4),
                        r4b[:, h * 4:(h + 1) * 4],
                    )
                    rcbs.append(rcb)

                def finish():
                    # emitted into the NEXT q-block's filler stream so the
                    # in-order PE never waits on the rcb DMA round-trip
                    bps = ps.tile([128, 512], dt.float32, tag="pp", bufs=2,
                                  name=f"bps_{b}_{qb}")
                    for h in range(NH):
                        nc.tensor.matmul(
                            bps[h * HD:(h + 1) * HD, :], ones_sb[:],
                            rcbs[h][:], start=True, stop=True,
                        )
                    for h in range(NH):
                        nc.vector.tensor_mul(
                            ot_sb[h * HD:(h + 1) * HD, q0:q0 + 512],
                            ms[h][0:HD, :], bps[h * HD:(h + 1) * HD, :],
                        )
                # insert a few filler slots in: late enough that the rcb DMA
                # round-trip (~2.5us) is done when the in-order PE reaches
                # the broadcast matmuls, early enough not to starve later
                # slots (ot is only read 4 q-blocks later).
                fifo.insert(min(3, len(fifo)), finish)

            # ---------------- emission schedule ----------------
            # DMA dispatch order = consumption order: the DMA queues are
            # FIFO, so wk must not sit behind all 4MB of batch-0 x.
            _wload(wk_ext, wk_sb)
            xkv = {0: load_x(0, "xtkv")}
            _wload(wq_ext, wq_sb)
            nc.sync.dma_start(bq_sb[:], bq_ext[:])
            _wload(wv_ext, wv_sb)
            nc.sync.dma_start(
                bvb_sb[:],
                bv_ext[:].rearrange("o (p f) -> o p f", p=1)
                .broadcast_to((1, 128, F)),
            )
            for rb in range(1, 4):
                xkv[rb] = load_x(rb, "xtkv")
            nc.sync.dma_start(wo_sb[:], wo_ext[:])
            nc.vector.memset(ones_sb[:], 1.0)
            nc.vector.memset(
                va_sb[:].rearrange("p (g d) -> p g d", d=VW)[:, :, HD:VW], 1.0
            )

            # upfront: projections for row-block 0 only; K/V for row-blocks
            # 1-3 ride the filler FIFO inside the first q-block (K(rb) is
            # consumed by S at jp=2rb, V(rb) by PV two jps later)
            p1_qk(0, xkv[0], wk_sb, kt_sb, None)
            p1_qk(0, xkv[0], wq_sb, qt_sb, bq_sb)
            for sub in range(4):
                p1_v(0, xkv[0], sub)
            for rb in range(1, 4):
                fifo.append(lambda rb=rb: p1_qk(rb, xkv[rb], wk_sb, kt_sb, None))
                for sub in range(4):
                    fifo.append(lambda rb=rb, s=sub: p1_v(rb, xkv[rb], s))
            xq = {1: load_x(1, "xtq")}
            xkv[4] = load_x(4, "xtkv")

            for g in range(NG):
                b, qb = g // NQB, g % NQB
                # enqueue fillers for this block (see module docstring);
                # x tiles resolve at closure run time so a FIFO backlog can
                # never use a tile before its load closure ran.
                if g + 1 < NG:   # Q projection for the next q-block
                    fifo.append(lambda rb=g + 1: p1_qk(rb, xq[rb], wq_sb, qt_sb, bq_sb))
                if g + 2 < NG:
                    def lq(rb=g + 2):
                        xq[rb] = load_x(rb, "xtq")
                    fifo.append(load_closure(lq))
                if g + 4 < NG:   # K/V projections for q-block g+4
                    rb = g + 4
                    fifo.append(lambda rb=rb: p1_qk(rb, xkv[rb], wk_sb, kt_sb, None))
                    for sub in range(4):
                        fifo.append(lambda rb=rb, s=sub: p1_v(rb, xkv[rb], s))
                if g + 5 < NG:
                    def lkv(rb=g + 5):
                        xkv[rb] = load_x(rb, "xtkv")
                    fifo.append(load_closure(lkv))
                if g >= 4:       # output projection of q-block g-4
                    for t in range(4 * (g - 4), 4 * (g - 4) + 4):
                        fifo.append(lambda t=t: p3_tile(t))
                if g >= 13:      # last batch: also its own previous q-block
                    for t in range(4 * (g - 1), 4 * (g - 1) + 4):
                        fifo.append(lambda t=t: p3_tile(t))
                p2_qblock(b, qb, pull2=(g == 0), last=(g == NG - 1))
            # tail: drain leftovers + output projection of the last q-block
            fill(len(fifo))
            for t in range(60, 64):
                p3_tile(t)
    return nc


_NC_CACHE = None


def _get_nc():
    global _NC_CACHE
    if _NC_CACHE is None:
        _NC_CACHE = build_bass()
    return _NC_CACHE


def make_in_maps(x, Wq, bq, Wk, bk, Wv, bv, Wo, bo):
    xt = np.ascontiguousarray(
        np.asarray(x, dtype=np.float32).reshape(R, EMB).astype(bf16).T
    )
    in_maps = []
    for c in range(NCORES):
        rows = slice(F * c, F * (c + 1))
        in_maps.append({
            "xt": xt,
            "wq": np.ascontiguousarray(np.asarray(Wq)[rows, :].T.astype(bf16)),
            "wk": np.ascontiguousarray(np.asarray(Wk)[rows, :].T.astype(bf16)),
            "wv": np.ascontiguousarray(np.asarray(Wv)[rows, :].T.astype(bf16)),
            "wo": np.ascontiguousarray(np.asarray(Wo)[:, rows].T.astype(bf16)),
            "bq": np.asarray(bq)[rows].reshape(F, 1).astype(np.float32),
            "bv": np.asarray(bv)[rows].reshape(1, F).astype(np.float32),
        })
    return in_maps


def gather(results, bo):
    acc = np.zeros((R, EMB), np.float32)
    for r in results:
        acc += r["out"].astype(np.float32)
    acc += np.asarray(bo, dtype=np.float32)
    return acc.reshape(B, T, EMB)


def kernel(x, Wq, bq, Wk, bk, Wv, bv, Wo, bo, _trace=False):
    nc = _get_nc()
    in_maps = make_in_maps(x, Wq, bq, Wk, bk, Wv, bv, Wo, bo)
    res = run_bass_kernel_spmd(nc, in_maps, list(range(NCORES)), trace=_trace)
    out = gather(res.results, bo)
    if _trace:
        kernel.last_result = res
    return out


# revision 78
# speedup vs baseline: 1.0511x; 1.0511x over previous
"""Multi-head self-attention (B=4, T=2048, C=1024, 16 heads x hd=64) on 8
Trainium2 NeuronCores.

Sharding: tensor-parallel over heads — each core owns 2 heads (128 of the
1024 channels): its slices of Wq/Wk/Wv rows and Wo columns. Every core reads
the full x (transposed + bf16-cast on host), computes Q^T/K^T (channel-major)
and V (token-major) for its heads, runs attention entirely from SBUF, then
produces a rank-128 partial of the output projection. The 8 partials are
summed on host (+ bo).

Per-core dataflow (all matmuls bf16 in / fp32 PSUM accumulate):
  phase 1: Q^T = Wq_c @ x^T (+bq), K^T = Wk_c @ x^T (bk dropped — it only
           shifts every score in a softmax row by a constant), V = x @ Wv_c^T
           token-major with a ones column appended per head (denominator
           trick) and bv folded into V (softmax weights sum to 1, so adding
           bv to every V row adds exactly bv to the output).
  phase 2: per (batch, 512-query block): S^T [128k, 1024(2 k-tiles)] per head
           via K^T-stationary matmuls (contraction d=64), one exp per k-tile
           pair on ScalarE (scale=1/8 folded in) -> P^T bf16, then
           O^T[65,512] += [V|1]^T P^T accumulated over k with K=128 matmuls.
           Normalize: VectorE copies pv->m (releasing the pv PSUM banks for
           the next q-block), the denominator row is DMA-reshaped to [128,4]
           so the reciprocal runs 128 DVE lanes wide, DMA'd back to [1,512]
           bf16, broadcast over 64 partitions with a K=1 ones matmul on the
           PE, then ot = m * bps. The broadcast+mul are deferred into the
           next q-block's filler stream so the in-order PE never waits on
           the DMA round-trip (ot is only read 4 q-blocks later); the last
           q-block instead computes 1/denom = exp(-ln denom) on the then-
           idle ScalarE (Ln and Exp share an activation table).
  phase 3: partial_out[128 rows, 1024] = O^T-slice-stationary matmuls against
           Wo_c^T; fp16 partials DMA'd out via the GpSimd DGE queue.

Scheduling: a global FIFO of small (~0.5-2us) filler closures is drained
inside phase 2's ACT-bound loops (2 pulls after the first two score groups,
1 after each later score/PV group). Per global query-block G the fillers
are the Q projection for G+1 (Q is only needed by its own query block, so
it rides one block ahead), K/V projections for G+4, and output-projection
tiles of G-4 (plus G-1 during the last batch, which has no projections
left). Only row-block 0's projections run before attention starts; K/V for
row-blocks 1-3 ride the FIFO inside the first q-block, just ahead of the
S/PV groups that consume them. x and weight loads are single-dispatch DMAs
(packets stripe across all 16 DMA engines regardless, and each dispatch
costs ~780ns of Sync-engine time), ordered so wk/wq never sit behind the
4MB of batch-0 x, and split in half so the first matmuls start early.

Run-to-run HW time varies ~±15us with the device's power-throttle state
(throttle_active_nc0_time_ns in the profile); compare configs on
exec_time - 0.5*throttle_active.
"""
import json

import numpy as np
import ml_dtypes

import concourse.bass as bass
import concourse.mybir as mybir
import concourse.tile as tile
from concourse.bass_utils import run_bass_kernel_spmd

bf16 = ml_dtypes.bfloat16
dt = mybir.dt

EMB = 1024
HEADS = 16
HD = 64
B = 4
T = 2048
R = B * T            # 8192 rows
NCORES = 8
F = EMB // NCORES    # 128 channels (2 heads) per core
NH = F // HD         # 2 heads per core
NKC = EMB // 128     # 8 contraction chunks for projections
NQB = T // 512       # 4 query blocks per batch
NJP = T // 256       # 8 k-tile PAIRS per batch
NG = R // 512        # 16 global query blocks
G = R // 128         # 64 global row/key tiles
VW = HD + 1          # 65: V head slice + ones column


# ---------------------------------------------------------------------------
# walrus in this container accepts only ONE sync-wait per instruction; split
# extra waits onto same-engine NoOps at BIR-serialization time.
_orig_to_json_bytes = bass.Bass.to_json_bytes


def _split_waits(data: bytes) -> bytes:
    d = json.loads(data)
    changed = False
    for f in d.get("functions", []):
        for blk in f.get("blocks", []):
            out = []
            for inst in blk.get("instructions", []):
                si = inst.get("sync_info")
                waits = (si or {}).get("on_wait") or []
                if len(waits) > 1:
                    changed = True
                    for i, w in enumerate(waits[:-1]):
                        out.append({
                            "debug": inst.get("debug", 0),
                            "engine": inst["engine"],
                            "ins": [], "outs": [],
                            "name": f"{inst['name']}_w{i}",
                            "opcode": "NoOp",
                            "sync_info": {"on_update": [], "on_wait": [w]},
                            "text_hint": "wait_split",
                        })
                    si["on_wait"] = waits[-1:]
                out.append(inst)
            blk["instructions"] = out
    return json.dumps(d).encode() if changed else data


def _to_json_bytes(self, *a, **k):
    return _split_waits(_orig_to_json_bytes(self, *a, **k))


bass.Bass.to_json_bytes = _to_json_bytes
# ---------------------------------------------------------------------------


def build_bass() -> bass.Bass:
    nc = bass.Bass()
    xt_ext = nc.declare_dram_parameter("xt", [EMB, R], dt.bfloat16, isOutput=False)
    wq_ext = nc.declare_dram_parameter("wq", [EMB, F], dt.bfloat16, isOutput=False)
    wk_ext = nc.declare_dram_parameter("wk", [EMB, F], dt.bfloat16, isOutput=False)
    wv_ext = nc.declare_dram_parameter("wv", [EMB, F], dt.bfloat16, isOutput=False)
    wo_ext = nc.declare_dram_parameter("wo", [F, EMB], dt.bfloat16, isOutput=False)
    bq_ext = nc.declare_dram_parameter("bq", [F, 1], dt.float32, isOutput=False)
    bv_ext = nc.declare_dram_parameter("bv", [1, F], dt.float32, isOutput=False)
    out_ext = nc.declare_dram_parameter("out", [R, EMB], dt.float16, isOutput=True)

    Exp = mybir.ActivationFunctionType.Exp

    with tile.TileContext(nc) as tc:
        with (
            tc.tile_pool(name="const", bufs=1) as cp,
            tc.tile_pool(name="res", bufs=1) as res,
            tc.tile_pool(name="xt", bufs=1) as xp,
            tc.tile_pool(name="pt", bufs=6) as ptp,
            tc.tile_pool(name="norm", bufs=2) as npl,
            tc.tile_pool(name="osb", bufs=3) as op,
            tc.tile_pool(name="ps", bufs=1, space="PSUM") as ps,
        ):
            # --- constants ---
            wq_sb = cp.tile([128, EMB], dt.bfloat16, tag="wq")
            wk_sb = cp.tile([128, EMB], dt.bfloat16, tag="wk")
            wv_sb = cp.tile([128, EMB], dt.bfloat16, tag="wv")
            wo_sb = cp.tile([128, EMB], dt.bfloat16, tag="wo")
            bq_sb = cp.tile([F, 1], dt.float32, tag="bq")
            bvb_sb = cp.tile([128, F], dt.float32, tag="bvb")
            ones_sb = cp.tile([1, HD], dt.bfloat16, tag="ones")

            def _wload(ext, tile_sb):
                nc.sync.dma_start(
                    tile_sb[:].rearrange("p (kc f) -> p kc f", f=F),
                    ext[:].rearrange("(kc p) f -> p kc f", p=128),
                )

            # --- residents ---
            qt_sb = res.tile([F, R], dt.bfloat16, tag="qt")
            kt_sb = res.tile([F, R], dt.bfloat16, tag="kt")
            ot_sb = res.tile([F, R], dt.bfloat16, tag="ot")
            va_sb = res.tile([128, G * NH * VW], dt.bfloat16, tag="va")

            # ---- x loads: two dispatches per 512-row block (the split lets
            # the first projection matmuls start after half the data) ----
            def load_x(rb, tag, bufs=4):
                xts = []
                for half in range(2):
                    xt = xp.tile([128, 4 * 512], dt.bfloat16,
                                 tag=f"{tag}{half}", bufs=bufs,
                                 name=f"{tag}{half}_{rb}")
                    nc.sync.dma_start(
                        xt[:].rearrange("p (kc f) -> p kc f", f=512),
                        xt_ext[half * 512:half * 512 + 512,
                               rb * 512:rb * 512 + 512]
                        .rearrange("(kc p) f -> p kc f", p=128),
                    )
                    xts.append(xt)
                return xts

            def xsl(xts, kc, lo, hi):
                base = (kc % 4) * 512
                return xts[kc // 4][:, base + lo:base + hi]

            # ---- projection emitters ----
            def p1_qk(rb, xt, w_sb, dst_sb, bias):
                r0 = rb * 512
                acc = ps.tile([128, 512], dt.float32, tag="pp", bufs=2,
                              name=f"prj_{rb}_{id(w_sb)}")
                for kc in range(NKC):
                    nc.tensor.matmul(
                        acc[:], w_sb[:, kc * F:(kc + 1) * F],
                        xsl(xt, kc, 0, 512),
                        start=(kc == 0), stop=(kc == NKC - 1),
                    )
                if bias is not None:
                    nc.vector.tensor_scalar_add(
                        dst_sb[:, r0:r0 + 512], acc[:], bias[:])
                else:
                    nc.vector.tensor_copy(dst_sb[:, r0:r0 + 512], acc[:])

            def p1_v(rb, xt, sub):
                g = rb * 4 + sub
                acc = ps.tile([128, F], dt.float32, tag="pp", bufs=2,
                              name=f"vprj_{g}")
                for kc in range(NKC):
                    nc.tensor.matmul(
                        acc[:],
                        xsl(xt, kc, sub * 128, (sub + 1) * 128),
                        wv_sb[:, kc * F:(kc + 1) * F],
                        start=(kc == 0), stop=(kc == NKC - 1),
                    )
                dst = va_sb[:, g * NH * VW:(g + 1) * NH * VW].rearrange(
                    "p (h d) -> p h d", d=VW
                )[:, :, 0:HD]
                nc.vector.tensor_add(
                    dst, acc[:].rearrange("p (h d) -> p h d", d=HD),
                    bvb_sb[:].rearrange("p (h d) -> p h d", d=HD),
                )

            # ---- phase-3 emitter (one 128-row tile); out DMA dispatched off
            # the otherwise-idle GpSimd DGE queue ----
            def p3_tile(g):
                o_sb = op.tile([128, EMB], dt.float16, tag="osb", name=f"o_{g}")
                for ch in range(2):
                    o_ps = ps.tile([128, 512], dt.float32, tag="pp", bufs=2,
                                   name=f"ops_{g}_{ch}")
                    nc.tensor.matmul(
                        o_ps[:],
                        ot_sb[:, g * 128:(g + 1) * 128],
                        wo_sb[:, ch * 512:(ch + 1) * 512],
                        start=True, stop=True,
                    )
                    nc.vector.tensor_copy(o_sb[:, ch * 512:(ch + 1) * 512], o_ps[:])
                nc.gpsimd.dma_start(out_ext[g * 128:(g + 1) * 128, :], o_sb[:])

            # ---- global filler FIFO ----
            # Closures marked is_load=True emit no PE work (DMA dispatch
            # only); fill() doesn't count them toward its quota so a slot
            # between two PE-gating points always gets real PE filler.
            fifo = []

            def load_closure(fn):
                fn.is_load = True
                return fn

            def fill(n=1):
                for _ in range(n):
                    if fifo:
                        fifo.pop(0)()

            # ---- phase-2 q-block with interleaved fillers ----
            def p2_qblock(b, qb, pull2=False, last=False):
                q0 = b * T + qb * 512
                pvs = {h: ps.tile([VW, 512], dt.float32, tag="pv", bufs=2,
                                  name=f"pv_{b}_{qb}_{h}")
                       for h in range(NH)}
                pts = {}

                def emit_st(jp, midfill=False):
                    k0 = b * T + jp * 256
                    for h in range(NH):
                        st = ps.tile([128, 1024], dt.float32, tag="st", bufs=2,
                                     name=f"st_{b}_{qb}_{jp}_{h}")
                        for half in range(2):
                            nc.tensor.matmul(
                                st[:, half * 512:(half + 1) * 512],
                                kt_sb[h * HD:(h + 1) * HD,
                                      k0 + half * 128:k0 + (half + 1) * 128],
                                qt_sb[h * HD:(h + 1) * HD, q0:q0 + 512],
                                start=True, stop=True,
                            )
                        pt = ptp.tile([128, 1024], dt.bfloat16, tag="pt",
                                      name=f"pt_{b}_{qb}_{jp}_{h}")
                        nc.scalar.activation(pt[:], st[:], Exp, scale=0.125)
                        pts[(jp, h)] = pt
                        if midfill and h == 0:
                            fill(1)

                def emit_pv(jp):
                    g0 = b * NJP * 2 + jp * 2
                    for h in range(NH):
                        pt = pts.pop((jp, h))
                        for half in range(2):
                            g = g0 + half
                            va = va_sb[:, g * NH * VW + h * VW:
                                       g * NH * VW + (h + 1) * VW]
                            nc.tensor.matmul(
                                pvs[h][:], va[:],
                                pt[:, half * 512:(half + 1) * 512],
                                start=(jp == 0 and half == 0),
                                stop=(jp == NJP - 1 and half == 1),
                            )

                for jp in range(NJP):
                    emit_st(jp)
                    fill(2 if (jp < 2 or pull2) else 1)
                    if jp > 1:
                        emit_pv(jp - 2)
                        fill(1)
                emit_pv(NJP - 2)
                fill(1)
                emit_pv(NJP - 1)
                # ---- normalize. The m copies (rows 0:65, incl the
                # denominator) release the pv PSUM banks early so the next
                # q-block's PV accumulation never waits on this chain. The
                # denominator rows are DMA-reshaped to [128,4] so the DVE
                # reciprocal runs 128 lanes wide (~0.2us, vs 3.3us on one
                # lane), DMA'd back to [1,512] bf16, broadcast over 64
                # partitions with a K=1 ones matmul, then ot = m * bps.
                # Everything past the m copies is slack: ot is only read by
                # p3_tile fillers 4 q-blocks later. ----
                if last:
                    # tail: 1/denom = exp(-ln denom) on the (now idle)
                    # ScalarE — shorter critical path than the DMA reshape
                    bps = ps.tile([128, 512], dt.float32, tag="pp", bufs=2,
                                  name=f"bps_{b}_{qb}")
                    ms = []
                    for h in range(NH):
                        m = npl.tile([VW, 512], dt.float32, tag="m", bufs=4,
                                     name=f"m_{b}_{qb}_{h}")
                        nc.vector.tensor_copy(m[:], pvs[h][:])
                        ms.append(m)
                    for h in range(NH):
                        ld = npl.tile([1, 512], dt.float32, tag="ld",
                                      name=f"ld_{b}_{qb}_{h}")
                        nc.scalar.activation(
                            ld[:], ms[h][HD:VW, :],
                            mybir.ActivationFunctionType.Ln)
                        rcb = npl.tile([1, 512], dt.bfloat16, tag="rcb",
                                       bufs=4, name=f"rcbL_{b}_{qb}_{h}")
                        nc.scalar.activation(
                            rcb[:], ld[:],
                            mybir.ActivationFunctionType.Exp, scale=-1.0)
                        nc.tensor.matmul(
                            bps[h * HD:(h + 1) * HD, :], ones_sb[:], rcb[:],
                            start=True, stop=True,
                        )
                    for h in range(NH):
                        nc.vector.tensor_mul(
                            ot_sb[h * HD:(h + 1) * HD, q0:q0 + 512],
                            ms[h][0:HD, :], bps[h * HD:(h + 1) * HD, :],
                        )
                    return
                ms = []
                d4 = npl.tile([128, 2 * 4], dt.float32, tag="d4",
                              name=f"d4_{b}_{qb}")
                for h in range(NH):
                    m = npl.tile([VW, 512], dt.float32, tag="m", bufs=4,
                                 name=f"m_{b}_{qb}_{h}")
                    nc.vector.tensor_copy(m[:], pvs[h][:])
                    ms.append(m)
                    nc.sync.dma_start(
                        d4[:, h * 4:(h + 1) * 4]
                        .rearrange("p (a c) -> p a c", c=4),
                        m[HD:VW, :].rearrange("p (a c) -> p a c", c=4),
                    )
                r4 = npl.tile([128, 2 * 4], dt.float32, tag="r4",
                              name=f"r4_{b}_{qb}")
                nc.vector.reciprocal(r4[:], d4[:])
                r4b = npl.tile([128, 2 * 4], dt.bfloat16, tag="r4b",
                               name=f"r4b_{b}_{qb}")
                with nc.allow_low_precision(reason="1/denom bf16: 2^-9 rel"):
                    nc.vector.tensor_copy(r4b[:], r4[:])
                rcbs = []
                for h in range(NH):
                    rcb = npl.tile([1, 512], dt.bfloat16, tag="rcb", bufs=4,
                                   name=f"rcb_{b}_{qb}_{h}")
                    nc.sync.dma_start(
                        rcb[:].rearrange("p (a c) -> p a c", c=4),
                        r4b[:, h * 4:(h + 1) * 4],
                    )
                    rcbs.append(rcb)

                def finish():
                    # emitted into the NEXT q-block's filler stream so the
                    # in-order PE never waits on the rcb DMA round-trip
                    bps = ps.tile([128, 512], dt.float32, tag="pp", bufs=2,
                                  name=f"bps_{b}_{qb}")
                    for h in range(NH):
                        nc.tensor.matmul(
                            bps[h * HD:(h + 1) * HD, :], ones_sb[:],
                            rcbs[h][:], start=True, stop=True,
                        )
                    for h in range(NH):
                        nc.vector.tensor_mul(
                            ot_sb[h * HD:(h + 1) * HD, q0:q0 + 512],
                            ms[h][0:HD, :], bps[h * HD:(h + 1) * HD, :],
                        )
                # insert a few filler slots in: late enough that the rcb DMA
                # round-trip (~2.5us) is done when the in-order PE reaches
                # the broadcast matmuls, early enough not to starve later
                # slots (ot is only read 4 q-blocks later).
                fifo.insert(min(3, len(fifo)), finish)

            # ---------------- emission schedule ----------------
            # DMA dispatch order = consumption order: the DMA queues are
            # FIFO, so wk must not sit behind all 4MB of batch-0 x.
            _wload(wk_ext, wk_sb)
            xkv = {0: load_x(0, "xtkv")}
            _wload(wq_ext, wq_sb)
            nc.sync.dma_start(bq_sb[:], bq_ext[:])
            _wload(wv_ext, wv_sb)
            nc.sync.dma_start(
                bvb_sb[:],
                bv_ext[:].rearrange("o (p f) -> o p f", p=1)
                .broadcast_to((1, 128, F)),
            )
            for rb in range(1, 4):
                xkv[rb] = load_x(rb, "xtkv")
            nc.sync.dma_start(wo_sb[:], wo_ext[:])
            nc.vector.memset(ones_sb[:], 1.0)
            nc.vector.memset(
                va_sb[:].rearrange("p (g d) -> p g d", d=VW)[:, :, HD:VW], 1.0
            )

            # upfront: projections for row-block 0 only; K/V for row-blocks
            # 1-3 ride the filler FIFO inside the first q-block (K(rb) is
            # consumed by S at jp=2rb, V(rb) by PV two jps later)
            p1_qk(0, xkv[0], wk_sb, kt_sb, None)
            p1_qk(0, xkv[0], wq_sb, qt_sb, bq_sb)
            for sub in range(4):
                p1_v(0, xkv[0], sub)
            for rb in range(1, 4):
                fifo.append(lambda rb=rb: p1_qk(rb, xkv[rb], wk_sb, kt_sb, None))
                for sub in range(4):
                    fifo.append(lambda rb=rb, s=sub: p1_v(rb, xkv[rb], s))
            xq = {1: load_x(1, "xtq")}
            xkv[4] = load_x(4, "xtkv")

            for g in range(NG):
                b, qb = g // NQB, g % NQB
                # enqueue fillers for this block (see module docstring);
                # x tiles resolve at closure run time so a FIFO backlog can
                # never use a tile before its load closure ran.
                if g + 1 < NG:   # Q projection for the next q-block
                    fifo.append(lambda rb=g + 1: p1_qk(rb, xq[rb], wq_sb, qt_sb, bq_sb))
                if g + 2 < NG:
                    def lq(rb=g + 2):
                        xq[rb] = load_x(rb, "xtq")
                    fifo.append(load_closure(lq))
                if g + 4 < NG:   # K/V projections for q-block g+4
                    rb = g + 4
                    fifo.append(lambda rb=rb: p1_qk(rb, xkv[rb], wk_sb, kt_sb, None))
                    for sub in range(4):
                        fifo.append(lambda rb=rb, s=sub: p1_v(rb, xkv[rb], s))
                if g + 5 < NG:
                    def lkv(rb=g + 5):
                        xkv[rb] = load_x(rb, "xtkv")
                    fifo.append(load_closure(lkv))
                if g >= 4:       # output projection of q-block g-4
                    for t in range(4 * (g - 4), 4 * (g - 4) + 4):
                        fifo.append(lambda t=t: p3_tile(t))
                if g >= 13:      # last batch: also its own previous q-block
                    for t in range(4 * (g - 1), 4 * (g - 1) + 4):
                        fifo.append(lambda t=t: p3_tile(t))
                p2_qblock(b, qb, pull2=(g == 0), last=(g == NG - 1))
            # tail: drain leftovers + output projection of the last q-block
            fill(len(fifo))
            for t in range(60, 64):
                p3_tile(t)
    return nc


_NC_CACHE = None


def _get_nc():
    global _NC_CACHE
    if _NC_CACHE is None:
        _NC_CACHE = build_bass()
    return _NC_CACHE


def make_in_maps(x, Wq, bq, Wk, bk, Wv, bv, Wo, bo):
    xt = np.ascontiguousarray(
        np.asarray(x, dtype=np.float32).reshape(R, EMB).astype(bf16).T
    )
    in_maps = []
    for c in range(NCORES):
        rows = slice(F * c, F * (c + 1))
        in_maps.append({
            "xt": xt,
            "wq": np.ascontiguousarray(np.asarray(Wq)[rows, :].T.astype(bf16)),
            "wk": np.ascontiguousarray(np.asarray(Wk)[rows, :].T.astype(bf16)),
            "wv": np.ascontiguousarray(np.asarray(Wv)[rows, :].T.astype(bf16)),
            "wo": np.ascontiguousarray(np.asarray(Wo)[:, rows].T.astype(bf16)),
            "bq": np.asarray(bq)[rows].reshape(F, 1).astype(np.float32),
            "bv": np.asarray(bv)[rows].reshape(1, F).astype(np.float32),
        })
    return in_maps


def gather(results, bo):
    acc = np.zeros((R, EMB), np.float32)
    for r in results:
        acc += r["out"].astype(np.float32)
    acc += np.asarray(bo, dtype=np.float32)
    return acc.reshape(B, T, EMB)


def kernel(x, Wq, bq, Wk, bk, Wv, bv, Wo, bo, _trace=False):
    nc = _get_nc()
    in_maps = make_in_maps(x, Wq, bq, Wk, bk, Wv, bv, Wo, bo)
    res = run_bass_kernel_spmd(nc, in_maps, list(range(NCORES)), trace=_trace)
    out = gather(res.results, bo)
    if _trace:
        kernel.last_result = res
    return out


# revision 79
# speedup vs baseline: 1.0733x; 1.0211x over previous
"""Multi-head self-attention (B=4, T=2048, C=1024, 16 heads x hd=64) on 8
Trainium2 NeuronCores.

Sharding: tensor-parallel over heads — each core owns 2 heads (128 of the
1024 channels): its slices of Wq/Wk/Wv rows and Wo columns. Every core reads
the full x (transposed + bf16-cast on host), computes Q^T/K^T (channel-major)
and V (token-major) for its heads, runs attention entirely from SBUF, then
produces a rank-128 partial of the output projection. The 8 partials are
summed on host (+ bo).

Per-core dataflow (all matmuls bf16 in / fp32 PSUM accumulate):
  phase 1: Q^T = Wq_c @ x^T (+bq), K^T = Wk_c @ x^T (bk dropped — it only
           shifts every score in a softmax row by a constant), V = x @ Wv_c^T
           token-major with a ones column appended per head (denominator
           trick) and bv folded into V (softmax weights sum to 1, so adding
           bv to every V row adds exactly bv to the output).
  phase 2: per (batch, 512-query block): S^T [128k, 1024(2 k-tiles)] per head
           via K^T-stationary matmuls (contraction d=64), one exp per k-tile
           pair on ScalarE (scale=1/8 folded in) -> P^T bf16, then
           O^T[65,512] += [V|1]^T P^T accumulated over k with K=128 matmuls.
           Normalize: VectorE copies pv->m (releasing the pv PSUM banks for
           the next q-block), the denominator row is DMA-reshaped to [128,4]
           so the reciprocal runs 128 DVE lanes wide, DMA'd back to [1,512]
           bf16, broadcast over 64 partitions with a K=1 ones matmul on the
           PE, then ot = m * bps. The broadcast+mul are deferred into the
           next q-block's filler stream so the in-order PE never waits on
           the DMA round-trip (ot is only read 4 q-blocks later); the last
           q-block instead computes 1/denom = exp(-ln denom) on the then-
           idle ScalarE (Ln and Exp share an activation table).
  phase 3: partial_out[128 rows, 1024] = O^T-slice-stationary matmuls against
           Wo_c^T; fp16 partials DMA'd out via the GpSimd DGE queue.

Scheduling: a global FIFO of small (~0.5-2us) filler closures is drained
inside phase 2's ACT-bound loops (2 pulls after the first two score groups,
1 after each later score/PV group). Per global query-block G the fillers
are the Q projection for G+1 (Q is only needed by its own query block, so
it rides one block ahead), K/V projections for G+4, and output-projection
tiles of G-4 (plus G-1 during the last batch, which has no projections
left). Only row-block 0's projections run before attention starts; K/V for
row-blocks 1-3 ride the FIFO inside the first q-block, just ahead of the
S/PV groups that consume them. x and weight loads are single-dispatch DMAs
(packets stripe across all 16 DMA engines regardless, and each dispatch
costs ~780ns of Sync-engine time), ordered so wk/wq never sit behind the
4MB of batch-0 x, and split in half so the first matmuls start early.

Run-to-run HW time varies ~±15us with the device's power-throttle state
(throttle_active_nc0_time_ns in the profile); compare configs on
exec_time - 0.5*throttle_active.
"""
import json

import numpy as np
import ml_dtypes

import concourse.bass as bass
import concourse.mybir as mybir
import concourse.tile as tile
from concourse.bass_utils import run_bass_kernel_spmd

bf16 = ml_dtypes.bfloat16
dt = mybir.dt

EMB = 1024
HEADS = 16
HD = 64
B = 4
T = 2048
R = B * T            # 8192 rows
NCORES = 8
F = EMB // NCORES    # 128 channels (2 heads) per core
NH = F // HD         # 2 heads per core
NKC = EMB // 128     # 8 contraction chunks for projections
NQB = T // 512       # 4 query blocks per batch
NJP = T // 256       # 8 k-tile PAIRS per batch
NG = R // 512        # 16 global query blocks
G = R // 128         # 64 global row/key tiles
VW = HD + 1          # 65: V head slice + ones column


# ---------------------------------------------------------------------------
# walrus in this container accepts only ONE sync-wait per instruction; split
# extra waits onto same-engine NoOps at BIR-serialization time.
_orig_to_json_bytes = bass.Bass.to_json_bytes


def _split_waits(data: bytes) -> bytes:
    d = json.loads(data)
    changed = False
    for f in d.get("functions", []):
        for blk in f.get("blocks", []):
            out = []
            for inst in blk.get("instructions", []):
                si = inst.get("sync_info")
                waits = (si or {}).get("on_wait") or []
                if len(waits) > 1:
                    changed = True
                    for i, w in enumerate(waits[:-1]):
                        out.append({
                            "debug": inst.get("debug", 0),
                            "engine": inst["engine"],
                            "ins": [], "outs": [],
                            "name": f"{inst['name']}_w{i}",
                            "opcode": "NoOp",
                            "sync_info": {"on_update": [], "on_wait": [w]},
                            "text_hint": "wait_split",
                        })
                    si["on_wait"] = waits[-1:]
                out.append(inst)
            blk["instructions"] = out
    return json.dumps(d).encode() if changed else data


def _to_json_bytes(self, *a, **k):
    return _split_waits(_orig_to_json_bytes(self, *a, **k))


bass.Bass.to_json_bytes = _to_json_bytes
# ---------------------------------------------------------------------------


def build_bass() -> bass.Bass:
    nc = bass.Bass()
    xt_ext = nc.declare_dram_parameter("xt", [EMB, R], dt.bfloat16, isOutput=False)
    wq_ext = nc.declare_dram_parameter("wq", [EMB, F], dt.bfloat16, isOutput=False)
    wk_ext = nc.declare_dram_parameter("wk", [EMB, F], dt.bfloat16, isOutput=False)
    wv_ext = nc.declare_dram_parameter("wv", [EMB, F], dt.bfloat16, isOutput=False)
    wo_ext = nc.declare_dram_parameter("wo", [F, EMB], dt.bfloat16, isOutput=False)
    bq_ext = nc.declare_dram_parameter("bq", [F, 1], dt.float32, isOutput=False)
    bv_ext = nc.declare_dram_parameter("bv", [1, F], dt.float32, isOutput=False)
    out_ext = nc.declare_dram_parameter("out", [R, EMB], dt.float16, isOutput=True)

    Exp = mybir.ActivationFunctionType.Exp

    with tile.TileContext(nc) as tc:
        with (
            tc.tile_pool(name="const", bufs=1) as cp,
            tc.tile_pool(name="res", bufs=1) as res,
            tc.tile_pool(name="xt", bufs=1) as xp,
            tc.tile_pool(name="pt", bufs=8) as ptp,
            tc.tile_pool(name="norm", bufs=2) as npl,
            tc.tile_pool(name="osb", bufs=3) as op,
            tc.tile_pool(name="ps", bufs=1, space="PSUM") as ps,
        ):
            # --- constants ---
            wq_sb = cp.tile([128, EMB], dt.bfloat16, tag="wq")
            wk_sb = cp.tile([128, EMB], dt.bfloat16, tag="wk")
            wv_sb = cp.tile([128, EMB], dt.bfloat16, tag="wv")
            wo_sb = cp.tile([128, EMB], dt.bfloat16, tag="wo")
            bq_sb = cp.tile([F, 1], dt.float32, tag="bq")
            bvb_sb = cp.tile([128, F], dt.float32, tag="bvb")
            ones_sb = cp.tile([1, HD], dt.bfloat16, tag="ones")

            def _wload(ext, tile_sb):
                # two dispatches so the first projection matmuls only wait
                # for half the weight bytes
                for half in range(2):
                    nc.sync.dma_start(
                        tile_sb[:, half * 4 * F:(half + 1) * 4 * F]
                        .rearrange("p (kc f) -> p kc f", f=F),
                        ext[half * 512:half * 512 + 512, :]
                        .rearrange("(kc p) f -> p kc f", p=128),
                    )

            # --- residents ---
            qt_sb = res.tile([F, R], dt.bfloat16, tag="qt")
            kt_sb = res.tile([F, R], dt.bfloat16, tag="kt")
            ot_sb = res.tile([F, R], dt.bfloat16, tag="ot")
            va_sb = res.tile([128, G * NH * VW], dt.bfloat16, tag="va")

            # ---- x loads: two dispatches per 512-row block (the split lets
            # the first projection matmuls start after half the data) ----
            def load_x(rb, tag, bufs=4):
                xts = []
                for half in range(2):
                    xt = xp.tile([128, 4 * 512], dt.bfloat16,
                                 tag=f"{tag}{half}", bufs=bufs,
                                 name=f"{tag}{half}_{rb}")
                    nc.sync.dma_start(
                        xt[:].rearrange("p (kc f) -> p kc f", f=512),
                        xt_ext[half * 512:half * 512 + 512,
                               rb * 512:rb * 512 + 512]
                        .rearrange("(kc p) f -> p kc f", p=128),
                    )
                    xts.append(xt)
                return xts

            def xsl(xts, kc, lo, hi):
                base = (kc % 4) * 512
                return xts[kc // 4][:, base + lo:base + hi]

            # ---- projection emitters ----
            def p1_qk(rb, xt, w_sb, dst_sb, bias):
                r0 = rb * 512
                acc = ps.tile([128, 512], dt.float32, tag="pp", bufs=2,
                              name=f"prj_{rb}_{id(w_sb)}")
                for kc in range(NKC):
                    nc.tensor.matmul(
                        acc[:], w_sb[:, kc * F:(kc + 1) * F],
                        xsl(xt, kc, 0, 512),
                        start=(kc == 0), stop=(kc == NKC - 1),
                    )
                if bias is not None:
                    nc.vector.tensor_scalar_add(
                        dst_sb[:, r0:r0 + 512], acc[:], bias[:])
                else:
                    nc.vector.tensor_copy(dst_sb[:, r0:r0 + 512], acc[:])

            def p1_v(rb, xt, sub):
                g = rb * 4 + sub
                acc = ps.tile([128, F], dt.float32, tag="pp", bufs=2,
                              name=f"vprj_{g}")
                for kc in range(NKC):
                    nc.tensor.matmul(
                        acc[:],
                        xsl(xt, kc, sub * 128, (sub + 1) * 128),
                        wv_sb[:, kc * F:(kc + 1) * F],
                        start=(kc == 0), stop=(kc == NKC - 1),
                    )
                dst = va_sb[:, g * NH * VW:(g + 1) * NH * VW].rearrange(
                    "p (h d) -> p h d", d=VW
                )[:, :, 0:HD]
                nc.vector.tensor_add(
                    dst, acc[:].rearrange("p (h d) -> p h d", d=HD),
                    bvb_sb[:].rearrange("p (h d) -> p h d", d=HD),
                )

            # ---- phase-3 emitter (one 128-row tile); out DMA dispatched off
            # the otherwise-idle GpSimd DGE queue ----
            def p3_tile(g):
                o_sb = op.tile([128, EMB], dt.float16, tag="osb", name=f"o_{g}")
                for ch in range(2):
                    o_ps = ps.tile([128, 512], dt.float32, tag="pp", bufs=2,
                                   name=f"ops_{g}_{ch}")
                    nc.tensor.matmul(
                        o_ps[:],
                        ot_sb[:, g * 128:(g + 1) * 128],
                        wo_sb[:, ch * 512:(ch + 1) * 512],
                        start=True, stop=True,
                    )
                    nc.vector.tensor_copy(o_sb[:, ch * 512:(ch + 1) * 512], o_ps[:])
                nc.gpsimd.dma_start(out_ext[g * 128:(g + 1) * 128, :], o_sb[:])

            # ---- global filler FIFO ----
            # Closures marked is_load=True emit no PE work (DMA dispatch
            # only); fill() doesn't count them toward its quota so a slot
            # between two PE-gating points always gets real PE filler.
            fifo = []

            def load_closure(fn):
                fn.is_load = True
                return fn

            def fill(n=1):
                for _ in range(n):
                    if fifo:
                        fifo.pop(0)()

            # ---- phase-2 q-block with interleaved fillers ----
            def p2_qblock(b, qb, pull2=False, last=False):
                q0 = b * T + qb * 512
                pvs = {h: ps.tile([VW, 512], dt.float32, tag="pv", bufs=2,
                                  name=f"pv_{b}_{qb}_{h}")
                       for h in range(NH)}
                pts = {}

                def emit_st(jp, midfill=False):
                    k0 = b * T + jp * 256
                    for h in range(NH):
                        st = ps.tile([128, 1024], dt.float32, tag="st", bufs=2,
                                     name=f"st_{b}_{qb}_{jp}_{h}")
                        for half in range(2):
                            nc.tensor.matmul(
                                st[:, half * 512:(half + 1) * 512],
                                kt_sb[h * HD:(h + 1) * HD,
                                      k0 + half * 128:k0 + (half + 1) * 128],
                                qt_sb[h * HD:(h + 1) * HD, q0:q0 + 512],
                                start=True, stop=True,
                            )
                        pt = ptp.tile([128, 1024], dt.bfloat16, tag="pt",
                                      name=f"pt_{b}_{qb}_{jp}_{h}")
                        nc.scalar.activation(pt[:], st[:], Exp, scale=0.125)
                        pts[(jp, h)] = pt
                        if midfill and h == 0:
                            fill(1)

                def emit_pv(jp):
                    g0 = b * NJP * 2 + jp * 2
                    for h in range(NH):
                        pt = pts.pop((jp, h))
                        for half in range(2):
                            g = g0 + half
                            va = va_sb[:, g * NH * VW + h * VW:
                                       g * NH * VW + (h + 1) * VW]
                            nc.tensor.matmul(
                                pvs[h][:], va[:],
                                pt[:, half * 512:(half + 1) * 512],
                                start=(jp == 0 and half == 0),
                                stop=(jp == NJP - 1 and half == 1),
                            )

                for jp in range(NJP):
                    emit_st(jp)
                    fill(2 if (jp < 2 or pull2) else 1)
                    if jp > 1:
                        emit_pv(jp - 2)
                        fill(1)
                emit_pv(NJP - 2)
                fill(1)
                emit_pv(NJP - 1)
                # ---- normalize. The m copies (rows 0:65, incl the
                # denominator) release the pv PSUM banks early so the next
                # q-block's PV accumulation never waits on this chain. The
                # denominator rows are DMA-reshaped to [128,4] so the DVE
                # reciprocal runs 128 lanes wide (~0.2us, vs 3.3us on one
                # lane), DMA'd back to [1,512] bf16, broadcast over 64
                # partitions with a K=1 ones matmul, then ot = m * bps.
                # Everything past the m copies is slack: ot is only read by
                # p3_tile fillers 4 q-blocks later. ----
                if last:
                    # tail: 1/denom = exp(-ln denom) on the (now idle)
                    # ScalarE — shorter critical path than the DMA reshape
                    bps = ps.tile([128, 512], dt.float32, tag="pp", bufs=2,
                                  name=f"bps_{b}_{qb}")
                    ms = []
                    for h in range(NH):
                        m = npl.tile([VW, 512], dt.float32, tag="m", bufs=4,
                                     name=f"m_{b}_{qb}_{h}")
                        nc.vector.tensor_copy(m[:], pvs[h][:])
                        ms.append(m)
                    for h in range(NH):
                        ld = npl.tile([1, 512], dt.float32, tag="ld",
                                      name=f"ld_{b}_{qb}_{h}")
                        nc.scalar.activation(
                            ld[:], ms[h][HD:VW, :],
                            mybir.ActivationFunctionType.Ln)
                        rcb = npl.tile([1, 512], dt.bfloat16, tag="rcb",
                                       bufs=4, name=f"rcbL_{b}_{qb}_{h}")
                        nc.scalar.activation(
                            rcb[:], ld[:],
                            mybir.ActivationFunctionType.Exp, scale=-1.0)
                        nc.tensor.matmul(
                            bps[h * HD:(h + 1) * HD, :], ones_sb[:], rcb[:],
                            start=True, stop=True,
                        )
                    for h in range(NH):
                        nc.vector.tensor_mul(
                            ot_sb[h * HD:(h + 1) * HD, q0:q0 + 512],
                            ms[h][0:HD, :], bps[h * HD:(h + 1) * HD, :],
                        )
                    return
                ms = []
                d4 = npl.tile([128, 2 * 4], dt.float32, tag="d4",
                              name=f"d4_{b}_{qb}")
                for h in range(NH):
                    m = npl.tile([VW, 512], dt.float32, tag="m", bufs=4,
                                 name=f"m_{b}_{qb}_{h}")
                    nc.vector.tensor_copy(m[:], pvs[h][:])
                    ms.append(m)
                    nc.sync.dma_start(
                        d4[:, h * 4:(h + 1) * 4]
                        .rearrange("p (a c) -> p a c", c=4),
                        m[HD:VW, :].rearrange("p (a c) -> p a c", c=4),
                    )
                r4 = npl.tile([128, 2 * 4], dt.float32, tag="r4",
                              name=f"r4_{b}_{qb}")
                nc.vector.reciprocal(r4[:], d4[:])
                r4b = npl.tile([128, 2 * 4], dt.bfloat16, tag="r4b",
                               name=f"r4b_{b}_{qb}")
                with nc.allow_low_precision(reason="1/denom bf16: 2^-9 rel"):
                    nc.vector.tensor_copy(r4b[:], r4[:])
                rcbs = []
                for h in range(NH):
                    rcb = npl.tile([1, 512], dt.bfloat16, tag="rcb", bufs=4,
                                   name=f"rcb_{b}_{qb}_{h}")
                    nc.sync.dma_start(
                        rcb[:].rearrange("p (a c) -> p a c", c=4),
                        r4b[:, h * 4:(h + 1) * 4],
                    )
                    rcbs.append(rcb)

                def finish():
                    # emitted into the NEXT q-block's filler stream so the
                    # in-order PE never waits on the rcb DMA round-trip
                    bps = ps.tile([128, 512], dt.float32, tag="pp", bufs=2,
                                  name=f"bps_{b}_{qb}")
                    for h in range(NH):
                        nc.tensor.matmul(
                            bps[h * HD:(h + 1) * HD, :], ones_sb[:],
                            rcbs[h][:], start=True, stop=True,
                        )
                    for h in range(NH):
                        nc.vector.tensor_mul(
                            ot_sb[h * HD:(h + 1) * HD, q0:q0 + 512],
                            ms[h][0:HD, :], bps[h * HD:(h + 1) * HD, :],
                        )
                # insert a few filler slots in: late enough that the rcb DMA
                # round-trip (~2.5us) is done when the in-order PE reaches
                # the broadcast matmuls, early enough not to starve later
                # slots (ot is only read 4 q-blocks later).
                fifo.insert(min(3, len(fifo)), finish)

            # ---------------- emission schedule ----------------
            # DMA dispatch order = consumption order: the DMA queues are
            # FIFO, so wk must not sit behind all 4MB of batch-0 x.
            _wload(wk_ext, wk_sb)
            xkv = {0: load_x(0, "xtkv")}
            _wload(wq_ext, wq_sb)
            nc.sync.dma_start(bq_sb[:], bq_ext[:])
            _wload(wv_ext, wv_sb)
            nc.sync.dma_start(
                bvb_sb[:],
                bv_ext[:].rearrange("o (p f) -> o p f", p=1)
                .broadcast_to((1, 128, F)),
            )
            for rb in range(1, 4):
                xkv[rb] = load_x(rb, "xtkv")
            nc.sync.dma_start(wo_sb[:], wo_ext[:])
            nc.vector.memset(ones_sb[:], 1.0)
            nc.vector.memset(
                va_sb[:].rearrange("p (g d) -> p g d", d=VW)[:, :, HD:VW], 1.0
            )

            # upfront: projections for row-block 0 only; K/V for row-blocks
            # 1-3 ride the filler FIFO inside the first q-block (K(rb) is
            # consumed by S at jp=2rb, V(rb) by PV two jps later)
            p1_qk(0, xkv[0], wk_sb, kt_sb, None)
            p1_qk(0, xkv[0], wq_sb, qt_sb, bq_sb)
            for sub in range(4):
                p1_v(0, xkv[0], sub)
            for rb in range(1, 4):
                fifo.append(lambda rb=rb: p1_qk(rb, xkv[rb], wk_sb, kt_sb, None))
                for sub in range(4):
                    fifo.append(lambda rb=rb, s=sub: p1_v(rb, xkv[rb], s))
            xq = {1: load_x(1, "xtq")}
            xkv[4] = load_x(4, "xtkv")

            for g in range(NG):
                b, qb = g // NQB, g % NQB
                # enqueue fillers for this block (see module docstring);
                # x tiles resolve at closure run time so a FIFO backlog can
                # never use a tile before its load closure ran.
                if g + 1 < NG:   # Q projection for the next q-block
                    fifo.append(lambda rb=g + 1: p1_qk(rb, xq[rb], wq_sb, qt_sb, bq_sb))
                if g + 2 < NG:
                    def lq(rb=g + 2):
                        xq[rb] = load_x(rb, "xtq")
                    fifo.append(load_closure(lq))
                if g + 4 < NG:   # K/V projections for q-block g+4
                    rb = g + 4
                    fifo.append(lambda rb=rb: p1_qk(rb, xkv[rb], wk_sb, kt_sb, None))
                    for sub in range(4):
                        fifo.append(lambda rb=rb, s=sub: p1_v(rb, xkv[rb], s))
                if g + 5 < NG:
                    def lkv(rb=g + 5):
                        xkv[rb] = load_x(rb, "xtkv")
                    fifo.append(load_closure(lkv))
                if g >= 4:       # output projection of q-block g-4
                    for t in range(4 * (g - 4), 4 * (g - 4) + 4):
                        fifo.append(lambda t=t: p3_tile(t))
                if g >= 13:      # last batch: also its own previous q-block
                    for t in range(4 * (g - 1), 4 * (g - 1) + 4):
                        fifo.append(lambda t=t: p3_tile(t))
                p2_qblock(b, qb, pull2=(g == 0), last=(g == NG - 1))
            # tail: drain leftovers + output projection of the last q-block
            fill(len(fifo))
            for t in range(60, 64):
                p3_tile(t)
    return nc


_NC_CACHE = None


def _get_nc():
    global _NC_CACHE
    if _NC_CACHE is None:
        _NC_CACHE = build_bass()
    return _NC_CACHE


def make_in_maps(x, Wq, bq, Wk, bk, Wv, bv, Wo, bo):
    xt = np.ascontiguousarray(
        np.asarray(x, dtype=np.float32).reshape(R, EMB).astype(bf16).T
    )
    in_maps = []
    for c in range(NCORES):
        rows = slice(F * c, F * (c + 1))
        in_maps.append({
            "xt": xt,
            "wq": np.ascontiguousarray(np.asarray(Wq)[rows, :].T.astype(bf16)),
            "wk": np.ascontiguousarray(np.asarray(Wk)[rows, :].T.astype(bf16)),
            "wv": np.ascontiguousarray(np.asarray(Wv)[rows, :].T.astype(bf16)),
            "wo": np.ascontiguousarray(np.asarray(Wo)[:, rows].T.astype(bf16)),
            "bq": np.asarray(bq)[rows].reshape(F, 1).astype(np.float32),
            "bv": np.asarray(bv)[rows].reshape(1, F).astype(np.float32),
        })
    return in_maps


def gather(results, bo):
    acc = np.zeros((R, EMB), np.float32)
    for r in results:
        acc += r["out"].astype(np.float32)
    acc += np.asarray(bo, dtype=np.float32)
    return acc.reshape(B, T, EMB)


def kernel(x, Wq, bq, Wk, bk, Wv, bv, Wo, bo, _trace=False):
    nc = _get_nc()
    in_maps = make_in_maps(x, Wq, bq, Wk, bk, Wv, bv, Wo, bo)
    res = run_bass_kernel_spmd(nc, in_maps, list(range(NCORES)), trace=_trace)
    out = gather(res.results, bo)
    if _trace:
        kernel.last_result = res
    return out
